# revision 14
# baseline (speedup 1.0000x reference)
"""Trainium2 Bass kernel for a char-LSTM (nn_CharsLstm).

Reference computation (B=4096 words, T=30 chars, D=512 emb, H=1024 hidden,
V=128 chars):
    xe = emb[x]                        # [B, T, D]
    scan over t: gates = xt @ W_ih.T + b_ih + h @ W_hh.T + b_hh
                 i, f, g, o = split(gates, 4)
                 c = sig(f)*c + sig(i)*tanh(g); h = sig(o)*tanh(c)
    return h                           # [B, H]

Strategy:
  - Data parallel: batch 4096 -> 8 cores x 512 words. No collectives.
  - Host folds embedding + input projection + both biases into one table:
        Wc = W_ih @ emb.T + (b_ih + b_hh)[:, None]    # [4H, V] = [4096, 128]
    so the x-path per step is a one-hot matmul with K=V=128 (4x fewer MACs
    than projecting D=512) and the bias comes for free (each one-hot column
    sums to 1).
  - Everything is kept transposed on-chip (batch in the free dim):
    gates.T [4H, 512], h.T/c.T [H, 512]. The elementwise stage then produces
    h.T chunks [128, 512] exactly in the layout the next step's matmul needs
    as its moving operand - no transposes anywhere.
  - Matmul inputs in fp16 (PE runs fp32 at 4 cycles/row; fp16/bf16 at 1
    cycle/row, and fp16's 10-bit mantissa fits W in +-1/32 and h in +-1
    easily: measured 8x less error than bf16, rel err ~1.7e-4). PSUM
    accumulation, c-state and all elementwise math stay fp32.
  - Per step: 32 output tiles [128p, 512] x (1 + 8) K-chunks = 288 matmuls
    = 147k PE cycles ~= 61 us at 1 col/cycle; ACT ~23 us and DVE ~22 us hide
    under PE. Cost-model total 1.89 ms (98% PE occupancy); slope-measured
    device time 1.3-1.9 ms.
"""

import numpy as np
import ml_dtypes

import concourse.bacc as bacc
import concourse.mybir as mybir
import concourse.tile as tile
from concourse.bass_utils import run_bass_kernel_spmd

B, T, D, H, V = 4096, 30, 512, 1024, 128
NCORES = 8
N = B // NCORES          # batch per core (matmul moving free dim)
KC = H // 128            # 8 K-chunks for the h-part
MC4 = 4 * H // 128       # 32 output row chunks
F32 = mybir.dt.float32
BF16 = mybir.dt.bfloat16
MMDT = mybir.dt.float16
MMNP = np.float16
SIG = mybir.ActivationFunctionType.Sigmoid
TANH = mybir.ActivationFunctionType.Tanh

_cached = {}


def build_kernel(n_steps=T, repeat=1, interleave=True, dma_split=8):
    nc = bacc.Bacc("TRN2", target_bir_lowering=False)

    # Host-prepared layouts (all contiguous, ready for single DMAs):
    #  whh  [128, KC*4096] fp16 : whh[p, k*4096+m] = W_hh[m, k*128+p]
    #  wemb [128, 4096]    fp16 : wemb[v, m] = Wc[m, v] (Wc = W_ih@emb.T + bias)
    #  oh   [128, T*512]   fp16 : oh[v, t*512+b] = (x[b, t] == v)
    #  h0t  [128, KC*512]  fp16 : h0t[p, k*512+b] = h0[b, k*128+p]
    #  c0t  [128, KC*512]  f32  : same layout as h0t
    #  out  [128, KC*512]  f32  : same layout (host inverts)
    whh_d = nc.dram_tensor("whh", [128, KC * 4096], MMDT, kind="ExternalInput")
    wemb_d = nc.dram_tensor("wemb", [128, 4 * H], MMDT, kind="ExternalInput")
    oh_d = nc.dram_tensor("oh", [128, n_steps * N], MMDT, kind="ExternalInput")
    h0_d = nc.dram_tensor("h0t", [128, KC * N], MMDT, kind="ExternalInput")
    c0_d = nc.dram_tensor("c0t", [128, KC * N], F32, kind="ExternalInput")
    out_d = nc.dram_tensor("out", [128, KC * N], F32, kind="ExternalOutput")

    with tile.TileContext(nc) as tc:
        with (
            tc.tile_pool(name="weights", bufs=1) as wpool,
            tc.tile_pool(name="state", bufs=2) as spool,
            tc.tile_pool(name="tmps", bufs=2) as tpool,
            tc.tile_pool(name="psum", bufs=8, space="PSUM") as ppool,
        ):
            # DMA emission order = consumption order: the first PE work is the
            # step-0 x-part (needs wemb + oh[0]) and the k-loop (needs h0 +
            # whh chunks in k order); ct is consumed by the first elementwise.
            wemb = wpool.tile([128, 4 * H], MMDT, tag="wemb")
            nc.sync.dma_start(out=wemb, in_=wemb_d[:, :])
            ht = spool.tile([128, KC * N], MMDT, tag="ht")
            nc.sync.dma_start(out=ht, in_=h0_d[:, :])
            oh = wpool.tile([128, n_steps * N], MMDT, tag="oh")
            n_oh_dma = max(1, dma_split // 4)
            for k in range(n_oh_dma):
                w = n_steps * N // n_oh_dma
                nc.sync.dma_start(out=oh[:, k * w:(k + 1) * w],
                                  in_=oh_d[:, k * w:(k + 1) * w])
            ct = wpool.tile([128, KC * N], F32, tag="ct")
            nc.sync.dma_start(out=ct, in_=c0_d[:, :])
            whh = wpool.tile([128, KC * 4096], MMDT, tag="whh")
            for k in range(dma_split):
                w = KC * 4096 // dma_split
                nc.sync.dma_start(out=whh[:, k * w:(k + 1) * w],
                                  in_=whh_d[:, k * w:(k + 1) * w])

            ht_fin = wpool.tile([128, KC * N], F32, tag="ht_fin")

            total = n_steps * repeat
            for s in range(total):
                t = s % n_steps
                last = s == total - 1
                ht_next = None if last else spool.tile([128, KC * N], MMDT, tag="ht")
                for j in range(KC):
                    # gates.T row chunks for this h-chunk: i, f, g, o
                    pts = [ppool.tile([128, N], F32, tag="ps", name=f"ps_{s}_{j}_{gi}")
                           for gi in range(4)]
                    if interleave:
                        # k-major across the 4 gate groups: the first MM that
                        # needs h[k=7] (produced by the previous step's last
                        # elementwise chunk) comes 32 MMs in, hiding the
                        # cross-step serial tail under already-issued work.
                        for gi in range(4):
                            m0 = gi * H + j * 128
                            nc.tensor.matmul(
                                pts[gi], wemb[:, m0:m0 + 128],
                                oh[:, t * N:(t + 1) * N],
                                start=True, stop=False,
                            )
                        for k in range(KC):
                            for gi in range(4):
                                m0 = gi * H + j * 128
                                nc.tensor.matmul(
                                    pts[gi],
                                    whh[:, k * 4096 + m0: k * 4096 + m0 + 128],
                                    ht[:, k * N:(k + 1) * N],
                                    start=False, stop=(k == KC - 1),
                                )
                    else:
                        for gi in range(4):
                            m0 = gi * H + j * 128
                            nc.tensor.matmul(
                                pts[gi], wemb[:, m0:m0 + 128],
                                oh[:, t * N:(t + 1) * N],
                                start=True, stop=False,
                            )
                            for k in range(KC):
                                nc.tensor.matmul(
                                    pts[gi],
                                    whh[:, k * 4096 + m0: k * 4096 + m0 + 128],
                                    ht[:, k * N:(k + 1) * N],
                                    start=False, stop=(k == KC - 1),
                                )

                    c_sl = ct[:, j * N:(j + 1) * N]
                    s_i = tpool.tile([128, N], F32, tag="s_i")
                    nc.scalar.activation(out=s_i, in_=pts[0], func=SIG)
                    s_g = tpool.tile([128, N], F32, tag="s_g")
                    nc.scalar.activation(out=s_g, in_=pts[2], func=TANH)
                    s_f = tpool.tile([128, N], F32, tag="s_f")
                    nc.scalar.activation(out=s_f, in_=pts[1], func=SIG)
                    s_o = tpool.tile([128, N], F32, tag="s_o")
                    nc.scalar.activation(out=s_o, in_=pts[3], func=SIG)

                    nc.vector.tensor_mul(s_i, s_i, s_g)      # sig(i)*tanh(g)
                    nc.vector.tensor_mul(c_sl, c_sl, s_f)    # sig(f)*c (in place)
                    nc.vector.tensor_add(c_sl, c_sl, s_i)    # c_new
                    s_tc = tpool.tile([128, N], F32, tag="s_tc")
                    nc.scalar.activation(out=s_tc, in_=c_sl, func=TANH)
                    h_sl = (ht_fin if last else ht_next)[:, j * N:(j + 1) * N]
                    nc.vector.tensor_mul(h_sl, s_o, s_tc)    # h = sig(o)*tanh(c)
                    if last:
                        # stream each finished chunk out while the remaining
                        # chunks still compute - hides most of the output DMA
                        # behind the final step's PE/ACT/DVE work
                        nc.sync.dma_start(out=out_d[:, j * N:(j + 1) * N],
                                          in_=h_sl)
                ht = ht_next

    nc.compile()
    return nc


def _prep_core_inputs(x, whh_t, wemb_t, h0, c0, core, n_steps=T):
    sl = slice(core * N, (core + 1) * N)
    x_c = np.asarray(x[sl])                      # [N, T] ints
    oh = (np.arange(V, dtype=np.int64)[:, None, None]
          == x_c.T[None, :n_steps, :])           # [V, T, N]
    oh = oh.reshape(V, n_steps * N).astype(MMNP)
    h0t = np.ascontiguousarray(
        h0[sl].reshape(N, KC, 128).transpose(2, 1, 0).reshape(128, KC * N)
    ).astype(MMNP)
    c0t = np.ascontiguousarray(
        c0[sl].reshape(N, KC, 128).transpose(2, 1, 0).reshape(128, KC * N)
    ).astype(np.float32)
    return {"whh": whh_t, "wemb": wemb_t, "oh": oh, "h0t": h0t, "c0t": c0t}


def kernel(x, emb, W_ih, W_hh, b_ih, b_hh, h0, c0, n_steps=T):
    x = np.asarray(x)
    emb = np.asarray(emb, dtype=np.float32)
    W_ih = np.asarray(W_ih, dtype=np.float32)
    W_hh = np.asarray(W_hh, dtype=np.float32)
    b_ih = np.asarray(b_ih, dtype=np.float32)
    b_hh = np.asarray(b_hh, dtype=np.float32)
    h0 = np.asarray(h0, dtype=np.float32)
    c0 = np.asarray(c0, dtype=np.float32)

    # Fold embedding + input projection + biases: Wc[m, v] = (W_ih @ emb.T + b)[m, v]
    wc = W_ih @ emb.T + (b_ih + b_hh)[:, None]           # [4H, V]
    wemb_t = np.ascontiguousarray(wc.T).astype(MMNP)  # [V, 4H]
    # whh[p, k*4096+m] = W_hh[m, k*128+p]
    whh_t = np.ascontiguousarray(
        W_hh.T.reshape(KC, 128, 4 * H).transpose(1, 0, 2).reshape(128, KC * 4096)
    ).astype(MMNP)

    key = n_steps
    if key not in _cached:
        _cached[key] = build_kernel(n_steps)
    nc = _cached[key]

    in_maps = [
        _prep_core_inputs(x, whh_t, wemb_t, h0, c0, core, n_steps)
        for core in range(NCORES)
    ]
    res = run_bass_kernel_spmd(nc, in_maps, core_ids=list(range(NCORES)))
    kernel.last_results = res

    out = np.empty((B, H), dtype=np.float32)
    for core in range(NCORES):
        ot = res.results[core]["out"]                    # [128, KC*N]
        out[core * N:(core + 1) * N] = (
            ot.reshape(128, KC, N).transpose(2, 1, 0).reshape(N, H)
        )
    return out
